# revision 1
# baseline (speedup 1.0000x reference)
"""Trainium2 Bass kernel for a dense transformer block.

Strategy: data-parallel over batch (8 batch elems -> 8 cores, no collectives).
Per core: x[1024, 1024] through LN1 -> qkv -> attention -> proj(+res) -> LN2 ->
fc1 -> gelu -> fc2(+res). Matmuls in bf16 with fp32 PSUM accumulation.
LayerNorm gamma/beta are folded into the following matmul's weights/bias on the
host (h = ln_core*g + b feeds only linear layers), and the attention scale
1/sqrt(hd) is folded into the q-part of the qkv weights.

Attention uses a transposed-scores layout: S^T[m, n] tiles come straight out of
the PE with keys (m) on partitions, exp() is applied on eviction (no max
subtraction needed: inputs are layernormed, |scores| is O(1)), and the P^T @ v
matmul contracts m on partitions.  A ones-column appended to v gives the
softmax denominator in the same PSUM tile, so normalization is a per-partition
reciprocal multiply during eviction.
"""

import numpy as np
import ml_dtypes

B = 8
T = 1024
D = 1024
H = 16
HD = D // H
FF = 4096
EPS = 1e-5
P = 128
N_CORES = 8

NT = T // P      # 8 token tiles
KD = D // P      # 8 contraction chunks over d
NF = 3 * D // P  # 24 qkv feature tiles
NFF = FF // P    # 32 ff feature tiles
HPF = P // HD    # 2 heads per 128-feature tile

_CACHE = {}


def _build_nc():
    from contextlib import ExitStack

    import concourse.bass as bass
    import concourse.mybir as mybir
    import concourse.tile as tile
    from concourse import bacc
    from concourse.masks import make_identity

    dt = mybir.dt
    f32, bf16 = dt.float32, dt.bfloat16
    AF = mybir.ActivationFunctionType
    ALU = mybir.AluOpType

    nc = bacc.Bacc("TRN2", target_bir_lowering=False, debug=False,
                   num_devices=N_CORES)

    xb = nc.dram_tensor("xb", [T, D], f32, kind="ExternalInput").ap()
    wqkvT = nc.dram_tensor("wqkvT", [D, 3 * D], bf16, kind="ExternalInput").ap()
    bqkv = nc.dram_tensor("bqkv", [3 * D], f32, kind="ExternalInput").ap()
    wprojT = nc.dram_tensor("wprojT", [D, D], bf16, kind="ExternalInput").ap()
    bproj = nc.dram_tensor("bproj", [D], f32, kind="ExternalInput").ap()
    wfc1T = nc.dram_tensor("wfc1T", [D, FF], bf16, kind="ExternalInput").ap()
    bfc1 = nc.dram_tensor("bfc1", [FF], f32, kind="ExternalInput").ap()
    wfc2T = nc.dram_tensor("wfc2T", [FF, D], bf16, kind="ExternalInput").ap()
    bfc2 = nc.dram_tensor("bfc2", [D], f32, kind="ExternalInput").ap()
    out = nc.dram_tensor("out", [T, D], f32, kind="ExternalOutput").ap()

    def bcast_ap(vec_ap, parts):
        # [N] dram vector -> [parts, N] partition-broadcast access pattern
        return bass.AP(tensor=vec_ap.tensor, offset=vec_ap.offset,
                       ap=[[0, parts]] + list(vec_ap.ap))

    with tile.TileContext(nc) as tc, ExitStack() as ctx:
        const = ctx.enter_context(tc.tile_pool(name="const", bufs=1))
        eps_t = const.tile([P, 1], f32)
        nc.vector.memset(eps_t, EPS)

        ident = const.tile([P, P], bf16)
        make_identity(nc, ident)
        ones_mat = const.tile([P, 64], bf16)
        nc.vector.memset(ones_mat, 1.0)

        bqkv_sb = const.tile([P, NF], f32)
        bfc1_sb = const.tile([P, NFF], f32)
        bproj_bc = const.tile([P, D], f32)
        bfc2_bc = const.tile([P, D], f32)

        # Global matmul PSUM pool (3 banks); phases open local pools for the rest
        ps_big = ctx.enter_context(tc.tile_pool(name="ps_big", bufs=4, space="PSUM"))

        wqkvT_r = wqkvT.rearrange("(k p) f -> p k f", p=P)
        wprojT_r = wprojT.rearrange("(k p) f -> p k f", p=P)
        wfc1T_r = wfc1T.rearrange("(k p) f -> p k f", p=P)
        wfc2T_r = wfc2T.rearrange("(k p) f -> p k f", p=P)

        def layernorm_to_T(src_pool, stat_pool, ps_tr, dst_tile, x_t, it):
            """x_t [P, D] f32 -> ln_core -> bf16 -> transposed into dst_tile."""
            st = stat_pool.tile([P, 2, 6], f32, name="st")
            xr = x_t.rearrange("p (s q) -> p s q", s=2)
            nc.vector.bn_stats(out=st[:, 0, :], in_=xr[:, 0, :])
            nc.vector.bn_stats(out=st[:, 1, :], in_=xr[:, 1, :])
            mv = stat_pool.tile([P, 2], f32, name="mv")
            nc.vector.bn_aggr(out=mv, in_=st)
            rstd = stat_pool.tile([P, 1], f32, name="rstd")
            nc.scalar.activation(out=rstd, in_=mv[:, 1:2], func=AF.Sqrt,
                                 bias=eps_t)
            nc.vector.reciprocal(out=rstd, in_=rstd)
            h_bf = src_pool.tile([P, D], bf16, name="h_bf")
            nc.vector.tensor_scalar(out=h_bf, in0=x_t, scalar1=mv[:, 0:1],
                                    scalar2=rstd, op0=ALU.subtract,
                                    op1=ALU.mult)
            for kd in range(KD):
                pt = ps_tr.tile([P, P], bf16, name="pt")
                nc.tensor.transpose(pt, h_bf[:, kd * P:(kd + 1) * P], ident)
                nc.vector.tensor_copy(
                    out=dst_tile[:, kd, it * P:(it + 1) * P], in_=pt)

        # Pool nesting is by lifetime (LIFO): x2 dies last, then oT, ...
        with tc.tile_pool(name="phX2", bufs=1) as phX2:
          x2 = phX2.tile([P, NT, D], f32)
          with tc.tile_pool(name="phOT", bufs=1) as phOT, \
               tc.tile_pool(name="phB", bufs=1) as phB:
            oT = phOT.tile([P, KD, T], bf16)
            qkvT = phB.tile([P, 2 * KD, T], bf16)
            # ---------------- Phase A: LN1 -> hT (DMA transposes) ----------
            with tc.tile_pool(name="phA", bufs=1) as phA:
              hT2 = [phA.tile([P, KD, T // 2], bf16, name="hTa"),
                     phA.tile([P, KD, T // 2], bf16, name="hTb")]
              with tc.tile_pool(name="xa", bufs=3) as xa_pool, \
                   tc.tile_pool(name="stat", bufs=4) as stat_pool, \
                   tc.tile_pool(name="pstrA", bufs=2, space="PSUM") as pstrA:
                for it in range(NT):
                    x_t = xa_pool.tile([P, D], f32, name="x_t")
                    nc.sync.dma_start(out=x_t, in_=xb[it * P:(it + 1) * P, :])
                    layernorm_to_T(xa_pool, stat_pool, pstrA, hT2[it // 4],
                                   x_t, it % 4)

              nc.sync.dma_start(out=bqkv_sb,
                                in_=bqkv.rearrange("(f p) -> p f", p=P))
              nc.sync.dma_start(out=bfc1_sb,
                                in_=bfc1.rearrange("(f p) -> p f", p=P))
              nc.sync.dma_start(out=bproj_bc, in_=bcast_ap(bproj, P))
              nc.sync.dma_start(out=bfc2_bc, in_=bcast_ap(bfc2, P))

              # ------- Phases B-D interleaved per head-pair: qkv tiles for
              # pair p, v transpose chunk p, then attention pair p.  Keeps the
              # PE dense (qkv matmuls overlap the ACT exp evictions of the
              # previous pair) so the HAM clock gate stays open. -------------
              with tc.tile_pool(name="phC", bufs=1) as phC:
                v_tok = phC.tile([P, NT, D], bf16)
                wv_sb = phC.tile([P, KD, D], bf16)
                for k in range(KD):
                    nc.sync.dma_start(out=wv_sb[:, k, :],
                                      in_=wqkvT_r[:, k, 2 * D:3 * D])
                with tc.tile_pool(name="wq", bufs=4) as wq_pool, \
                     tc.tile_pool(name="ptp", bufs=3) as pt_pool, \
                     tc.tile_pool(name="attn_sm", bufs=2) as sm_pool, \
                     tc.tile_pool(name="ps_op", bufs=2, space="PSUM") as ps_op, \
                     tc.tile_pool(name="ps_bcp", bufs=2, space="PSUM") as ps_bcp:
                    for p in range(H // HPF):
                        # --- q/k feature tiles p, KD + p ---
                        for ft in (p, KD + p):
                            w_t = wq_pool.tile([P, KD, P], bf16, name="w_t")
                            nc.sync.dma_start(
                                out=w_t, in_=wqkvT_r[:, :, ft * P:(ft + 1) * P])
                            for nh in range(2):
                                ps = ps_big.tile([P, 512], f32, name="ps_mm")
                                for k in range(KD):
                                    nc.tensor.matmul(
                                        ps, w_t[:, k, :],
                                        hT2[nh][:, k, :],
                                        start=(k == 0), stop=(k == KD - 1))
                                nc.vector.tensor_scalar(
                                    out=qkvT[:, ft, nh * 512:(nh + 1) * 512],
                                    in0=ps, scalar1=bqkv_sb[:, ft:ft + 1],
                                    scalar2=None, op0=ALU.add)
                        # --- v token-major, a 512-wide feature chunk every
                        # 4th pair (v_bias is folded into the proj bias) ---
                        if p % 4 == 0:
                            vsl = slice((p // 4) * 512, (p // 4) * 512 + 512)
                            for tt in range(NT):
                                ps = ps_big.tile([P, 512], f32, name="ps_mm")
                                hTh = hT2[tt // 4]
                                to = (tt % 4) * P
                                for k in range(KD):
                                    nc.tensor.matmul(
                                        ps, hTh[:, k, to:to + P],
                                        wv_sb[:, k, vsl],
                                        start=(k == 0), stop=(k == KD - 1))
                                nc.vector.tensor_copy(
                                    out=v_tok[:, tt, vsl], in_=ps)
                        # --- attention pair p ---
                        ft_q = p
                        ft_k = KD + p
                        PTs = []
                        bc_tiles = [
                            ps_bcp.tile([P, 512], f32, name="ps_bc"),
                            ps_bcp.tile([P, 512], f32, name="ps_bc"),
                        ]
                        for j in range(HPF):
                            po = j * HD
                            PT = pt_pool.tile([P, NT, T], bf16, name="PT")
                            PTs.append(PT)
                            for ch in range(2):
                                sl = slice(ch * 512, (ch + 1) * 512)
                                for mt in range(NT):
                                    ps = ps_big.tile([P, 512], f32,
                                                     name="ps_mm")
                                    nc.tensor.matmul(
                                        ps,
                                        qkvT[po:po + HD, ft_k,
                                             mt * P:(mt + 1) * P],
                                        qkvT[po:po + HD, ft_q, sl],
                                        start=True, stop=True)
                                    nc.scalar.activation(
                                        out=PT[:, mt, sl], in_=ps,
                                        func=AF.Exp)
                                # softmax denominator: 6 bf16 pair-adds on
                                # DVE/GpSimd, then a 2-matmul ones-chain
                                # broadcasts the column sums into this head's
                                # 64 partitions of the bc psum tile.
                                u0 = sm_pool.tile([P, 512], bf16, name="u0")
                                u1 = sm_pool.tile([P, 512], bf16, name="u1")
                                uv = sm_pool.tile([P, 512], bf16, name="uv")
                                nc.any.tensor_tensor(
                                    out=u0, in0=PT[:, 0, sl], in1=PT[:, 1, sl],
                                    op=ALU.add)
                                nc.any.tensor_tensor(
                                    out=uv, in0=PT[:, 2, sl], in1=PT[:, 3, sl],
                                    op=ALU.add)
                                nc.any.tensor_tensor(
                                    out=u0, in0=u0, in1=uv, op=ALU.add)
                                nc.any.tensor_tensor(
                                    out=u1, in0=PT[:, 4, sl], in1=PT[:, 5, sl],
                                    op=ALU.add)
                                nc.any.tensor_tensor(
                                    out=uv, in0=PT[:, 6, sl], in1=PT[:, 7, sl],
                                    op=ALU.add)
                                nc.any.tensor_tensor(
                                    out=u1, in0=u1, in1=uv, op=ALU.add)
                                nc.any.tensor_tensor(
                                    out=u0, in0=u0, in1=u1, op=ALU.add)
                                nc.tensor.matmul(
                                    bc_tiles[ch][po:po + HD, :],
                                    ones_mat[:, 0:HD], u0,
                                    start=True, stop=True)
                        # AV: o^T[feat, tok] accumulated over m chunks
                        for ch in range(2):
                            sl = slice(ch * 512, (ch + 1) * 512)
                            ps_o = ps_op.tile([P, 512], f32, name="ps_o")
                            for j in range(HPF):
                                po = j * HD
                                hf = (HPF * p + j) * HD
                                for mc in range(NT):
                                    nc.tensor.matmul(
                                        ps_o[po:po + HD, :],
                                        v_tok[:, mc, hf:hf + HD],
                                        PTs[j][:, mc, sl],
                                        start=(mc == 0), stop=(mc == NT - 1))
                            rec_sb = sm_pool.tile([P, 512], f32,
                                                  name="rec_sb")
                            nc.vector.reciprocal_approx_fast(
                                out=rec_sb, in_=bc_tiles[ch])
                            nc.vector.tensor_tensor(
                                out=oT[:, p, sl], in0=ps_o, in1=rec_sb,
                                op=ALU.mult)

            # -------- Phase E2: proj + residual -> x2 (qkvT/v/o freed) ------
            if True:
                with tc.tile_pool(name="wp", bufs=1) as wp_pool, \
                     tc.tile_pool(name="xr", bufs=2) as xr_pool:
                    wp_sb = wp_pool.tile([P, KD, D], bf16)
                    for k in range(KD):
                        nc.sync.dma_start(out=wp_sb[:, k, :],
                                          in_=wprojT_r[:, k, :])
                    for tt in range(NT):
                        x_r = xr_pool.tile([P, D], f32, name="x_r")
                        nc.sync.dma_start(out=x_r,
                                          in_=xb[tt * P:(tt + 1) * P, :])
                        for dh in range(2):
                            sl = slice(dh * 512, (dh + 1) * 512)
                            ps = ps_big.tile([P, 512], f32, name="ps_mm")
                            for k in range(KD):
                                nc.tensor.matmul(
                                    ps, oT[:, k, tt * P:(tt + 1) * P],
                                    wp_sb[:, k, sl],
                                    start=(k == 0), stop=(k == KD - 1))
                            nc.vector.tensor_tensor(
                                out=x2[:, tt, sl], in0=ps,
                                in1=bproj_bc[:, sl], op=ALU.add)
                            nc.vector.tensor_tensor(
                                out=x2[:, tt, sl], in0=x2[:, tt, sl],
                                in1=x_r[:, sl], op=ALU.add)

          # ---------------- Phase F: LN2 -> h2T ----------------
          with tc.tile_pool(name="phG2", bufs=1) as phG2:
            aT = phG2.tile([P, NFF, T], bf16)
            with tc.tile_pool(name="phF", bufs=1) as phF:
                h2T2 = [phF.tile([P, KD, T // 2], bf16, name="h2Ta"),
                        phF.tile([P, KD, T // 2], bf16, name="h2Tb")]
                with tc.tile_pool(name="xf", bufs=3) as xf_pool, \
                     tc.tile_pool(name="stat2", bufs=4) as stat2_pool, \
                     tc.tile_pool(name="pstrF", bufs=2, space="PSUM") as pstrF:
                    for it in range(NT):
                        layernorm_to_T(xf_pool, stat2_pool, pstrF,
                                       h2T2[it // 4], x2[:, it, :], it % 4)

                # ---------------- Phase G: fc1 + gelu -> aT ----------------
                with tc.tile_pool(name="w1", bufs=3) as w1_pool:
                    for ff in range(NFF):
                        w_t = w1_pool.tile([P, KD, P], bf16, name="w1_t")
                        nc.sync.dma_start(
                            out=w_t, in_=wfc1T_r[:, :, ff * P:(ff + 1) * P])
                        for nh in range(2):
                            ps = ps_big.tile([P, 512], f32, name="ps_mm")
                            for k in range(KD):
                                nc.tensor.matmul(
                                    ps, w_t[:, k, :],
                                    h2T2[nh][:, k, :],
                                    start=(k == 0), stop=(k == KD - 1))
                            nc.scalar.activation(
                                out=aT[:, ff, nh * 512:(nh + 1) * 512],
                                in_=ps, func=AF.Gelu,
                                bias=bfc1_sb[:, ff:ff + 1])

            # ---------------- Phase H: fc2 + residual -> out ----------
            if True:
                with tc.tile_pool(name="w2", bufs=2) as w2_pool, \
                     tc.tile_pool(name="yb", bufs=3) as y_pool:
                    for dh in range(2):
                        sl = slice(dh * 512, (dh + 1) * 512)
                        w2_t = w2_pool.tile([P, NFF, 512], bf16, name="w2_t")
                        for k in range(NFF):
                            nc.sync.dma_start(out=w2_t[:, k, :],
                                              in_=wfc2T_r[:, k, sl])
                        for tt in range(NT):
                            ps = ps_big.tile([P, 512], f32, name="ps_mm")
                            for k in range(NFF):
                                nc.tensor.matmul(
                                    ps, aT[:, k, tt * P:(tt + 1) * P],
                                    w2_t[:, k, :],
                                    start=(k == 0), stop=(k == NFF - 1))
                            y_sb = y_pool.tile([P, 512], f32, name="y_sb")
                            nc.vector.tensor_tensor(
                                out=y_sb, in0=ps, in1=bfc2_bc[:, sl],
                                op=ALU.add)
                            nc.vector.tensor_tensor(
                                out=y_sb, in0=y_sb, in1=x2[:, tt, sl],
                                op=ALU.add)
                            nc.sync.dma_start(
                                out=out[tt * P:(tt + 1) * P, sl], in_=y_sb)

    nc.compile()
    return nc


def _prep_host_inputs(x, ln1_g, ln1_b, ln2_g, ln2_b, qkv_w, q_bias, v_bias,
                      proj_w, proj_b, fc1_w, fc1_b, fc2_w, fc2_b):
    f32 = np.float32
    bf16 = ml_dtypes.bfloat16
    x = np.asarray(x, f32)
    ln1_g = np.asarray(ln1_g, f32)
    ln1_b = np.asarray(ln1_b, f32)
    ln2_g = np.asarray(ln2_g, f32)
    ln2_b = np.asarray(ln2_b, f32)
    qkv_w = np.asarray(qkv_w, f32)
    q_bias = np.asarray(q_bias, f32)
    v_bias = np.asarray(v_bias, f32)
    proj_w = np.asarray(proj_w, f32)
    proj_b = np.asarray(proj_b, f32)
    fc1_w = np.asarray(fc1_w, f32)
    fc1_b = np.asarray(fc1_b, f32)
    fc2_w = np.asarray(fc2_w, f32)
    fc2_b = np.asarray(fc2_b, f32)

    scale = HD ** (-0.5)
    # v_bias is a constant per-feature shift of o (softmax rows sum to 1),
    # so it folds into the proj bias: proj_b += proj_w @ v_bias.  The v part
    # of the qkv bias used on-chip is qkv_w_v @ ln1_b only.
    qkv_bias = np.concatenate(
        [q_bias, np.zeros_like(v_bias), np.zeros_like(v_bias)])
    wqkv = qkv_w * ln1_g[None, :]
    bqkv = qkv_w @ ln1_b + qkv_bias
    wqkv = wqkv.copy()
    wqkv[:D] *= scale
    bqkv[:D] *= scale
    proj_b = proj_b + proj_w @ (qkv_w[2 * D:] @ ln1_b + v_bias)

    wfc1 = fc1_w * ln2_g[None, :]
    bfc1 = fc1_w @ ln2_b + fc1_b

    shared = {
        "wqkvT": np.ascontiguousarray(wqkv.T).astype(bf16),
        "bqkv": np.ascontiguousarray(bqkv, f32),
        "wprojT": np.ascontiguousarray(proj_w.T).astype(bf16),
        "bproj": np.ascontiguousarray(proj_b, f32),
        "wfc1T": np.ascontiguousarray(wfc1.T).astype(bf16),
        "bfc1": np.ascontiguousarray(bfc1, f32),
        "wfc2T": np.ascontiguousarray(fc2_w.T).astype(bf16),
        "bfc2": np.ascontiguousarray(fc2_b, f32),
    }
    in_maps = [dict(shared, xb=np.ascontiguousarray(x[i]))
               for i in range(N_CORES)]
    return in_maps


def kernel(**inputs):
    from concourse.bass_utils import run_bass_kernel_spmd

    if "nc" not in _CACHE:
        _CACHE["nc"] = _build_nc()
    nc = _CACHE["nc"]
    in_maps = _prep_host_inputs(**inputs)
    res = run_bass_kernel_spmd(nc, in_maps, core_ids=list(range(N_CORES)),
                               trace=False)
    return np.stack([res.results[i]["out"] for i in range(N_CORES)], axis=0)


if __name__ == "__main__":
    rng = np.random.default_rng(0)
    ins = {
        "x": rng.standard_normal((B, T, D)).astype(np.float32),
        "ln1_g": np.ones(D, np.float32), "ln1_b": np.zeros(D, np.float32),
        "ln2_g": np.ones(D, np.float32), "ln2_b": np.zeros(D, np.float32),
        "qkv_w": (rng.uniform(-1, 1, (3 * D, D)) / 32).astype(np.float32),
        "q_bias": np.zeros(D, np.float32), "v_bias": np.zeros(D, np.float32),
        "proj_w": (rng.uniform(-1, 1, (D, D)) / 32).astype(np.float32),
        "proj_b": np.zeros(D, np.float32),
        "fc1_w": (rng.uniform(-1, 1, (FF, D)) / 32).astype(np.float32),
        "fc1_b": np.zeros(FF, np.float32),
        "fc2_w": (rng.uniform(-1, 1, (D, FF)) / 64).astype(np.float32),
        "fc2_b": np.zeros(D, np.float32),
    }
    y = kernel(**ins)
    print("out", y.shape, y.dtype, np.abs(y).max())

